# revision 1
# baseline (speedup 1.0000x reference)
"""DEMA (double exponential moving average) Trainium2 kernel.

Math: the per-lane recurrence
    s_t = a*x_t + (1-a)*(s_{t-1} + b_{t-1})
    b_t = B*(s_t - s_{t-1}) + (1-B)*b_{t-1}
is linear time-invariant: z_t = A z_{t-1} + c x_t with z = (s, b).
A chunk of K=126 timesteps is computed as ONE 128x128 @ 128xF matmul:
contraction rows 0..125 hold the chunk's x inputs, rows 126..127 hold the
incoming state (s, b); output rows 0..125 are the chunk's s outputs, rows
126..127 the outgoing state.  The transition matrix G is constant, so the
whole scan is 33 matmuls per (batch, core) with only a [2, F] state copy
serializing consecutive chunks of the same batch.

Sharding: pure data parallel over batch, 4 batches per core x 8 cores.

Engines: SP ring = input DMAs, ACT ring = output DMAs + PSUM->SBUF output
copies, PE = matmuls, DVE = state copies.  All synchronization is explicit
single-wait semaphores (this toolchain allows at most one sem wait per
instruction, which is why the Tile framework is not used).  DMA completion
sems are per buffer-slot so at most one group's DMAs are in flight per sem
(completions of concurrent DMAs are unordered).
"""

import math
from contextlib import ExitStack

import numpy as np

import concourse.bass as bass
from concourse import mybir
from concourse.bass_utils import run_bass_kernel_spmd

ALPHA = 0.3
BETA = 0.1

B, T, F = 32, 4096, 512
NCORES = 8
BLOC = B // NCORES
K = 126  # outputs per chunk (contraction 126 x-rows + 2 state rows = 128)
G = 4    # chunks per grouped DMA (~1MB per DMA)

F32 = mybir.dt.float32


def _build_gmats(dtype=np.float32):
    """Return (G0T, GT): transposed transition matrices, [128,128] each.

    Layout ("state-first"): compute engines require start partition 0, so
    the carried state lives in rows 0..1 and the s outputs in rows 2..127.

    GT (steady chunks):  rhs rows: 0..1 = (s, b) state, 2..127 = 126 x
    inputs.  out rows: 0..1 = outgoing (s, b) state, 2..127 = s outputs.
    G0T (chunk 0): rhs rows 0..127 = x_0..x_127 (state init s_0 = x_0,
    b_0 = x_1 - x_0 is folded into the matrix); out rows like GT.
    """
    A = np.array(
        [[1 - ALPHA, 1 - ALPHA], [-ALPHA * BETA, BETA * (1 - ALPHA) + 1 - BETA]],
        dtype=np.float64,
    )
    c = np.array([ALPHA, ALPHA * BETA], dtype=np.float64)
    P = [np.eye(2)]
    for _ in range(K + 1):
        P.append(P[-1] @ A)
    Ac = [P[k] @ c for k in range(K + 1)]

    # canonical layout first: rows/cols 0..125 = outputs/x-inputs, 126..127 = state
    Gm = np.zeros((128, 128), dtype=np.float64)
    for i in range(K):  # output s_{t0+1+i}
        for j in range(i + 1):
            Gm[i, j] = Ac[i - j][0]
        Gm[i, 126] = P[i + 1][0, 0]
        Gm[i, 127] = P[i + 1][0, 1]
    Gm[126, :] = Gm[125, :]
    for j in range(K):
        Gm[127, j] = Ac[K - 1 - j][1]
    Gm[127, 126] = P[K][1, 0]
    Gm[127, 127] = P[K][1, 1]

    G0 = np.zeros((128, 128), dtype=np.float64)
    for i in range(K):  # output s_{i+1}
        t = i + 1
        G0[i, 0] = P[t][0, 0] - P[t][0, 1]
        G0[i, 1] = P[t][0, 1] + Ac[t - 1][0]
        for u in range(2, t + 1):
            G0[i, u] = Ac[t - u][0]
    G0[126, :] = G0[125, :]
    G0[127, 0] = P[K][1, 0] - P[K][1, 1]
    G0[127, 1] = P[K][1, 1] + Ac[K - 1][1]
    for u in range(2, K + 1):
        G0[127, u] = Ac[K - u][1]

    # permute to state-first layout
    perm = np.empty(128, dtype=np.int64)  # perm[old] = new
    perm[:126] = np.arange(2, 128)
    perm[126] = 0
    perm[127] = 1
    inv = np.argsort(perm)
    Gm2 = Gm[inv][:, inv]        # rows and cols permuted
    G02 = G0[inv][:, :]          # only output rows permuted (input is raw x)
    return np.ascontiguousarray(G02.T, dtype=dtype), np.ascontiguousarray(Gm2.T, dtype=dtype)


def build_nc(bloc=BLOC, t=T, f=F, debug_skip=(), grp=None):
    nc = bass.Bass()
    st = ExitStack()
    nc._dema_exitstack = st  # keep sbuf/psum allocations alive

    if grp is None:
        grp = G
    nch = math.ceil((t - 1) / K)  # total chunks (chunk 0 special)
    # groups of steady-state chunks 1..nch-1
    groups = []
    cc = 1
    while cc < nch:
        groups.append(list(range(cc, min(cc + grp, nch))))
        cc += grp
    ngrp = len(groups)

    def rows_of(cc):
        return min(K, (t - 1) - K * cc)

    x = nc.dram_tensor("x", [bloc, t, f], F32, kind="ExternalInput")
    gw = nc.dram_tensor("gw", [2, 128, 128], F32, kind="ExternalInput")
    out = nc.dram_tensor("out", [bloc, t, f], F32, kind="ExternalOutput")

    ent = st.enter_context
    wt = ent(nc.sbuf_tensor("wt", [128, 2, 128], F32))
    rhs0 = [ent(nc.sbuf_tensor(f"rhs0_{b}", [128, f], F32)) for b in range(bloc)]
    ot0 = [ent(nc.sbuf_tensor(f"ot0_{b}", [128, f], F32)) for b in range(bloc)]
    grhs = [
        [ent(nc.sbuf_tensor(f"grhs_{b}_{s}", [128, grp, f], F32)) for s in range(2)]
        for b in range(bloc)
    ]
    gout = [
        [ent(nc.sbuf_tensor(f"gout_{b}_{s}", [128, grp, f], F32)) for s in range(2)]
        for b in range(bloc)
    ]
    ps = [ent(nc.psum_tensor(f"ps{i}", [128, f], F32)) for i in range(2 * bloc)]

    s_w = nc.alloc_semaphore("s_w")
    s_in0 = [nc.alloc_semaphore(f"s_in0_{b}") for b in range(bloc)]
    s_ing = [
        [nc.alloc_semaphore(f"s_ing{b}_{s}") for s in range(2)] for b in range(bloc)
    ]
    s_out0 = [nc.alloc_semaphore(f"s_out0_{b}") for b in range(bloc)]
    s_og = [
        [nc.alloc_semaphore(f"s_og{b}_{s}") for s in range(2)] for b in range(bloc)
    ]
    s_mm = [nc.alloc_semaphore(f"s_mm{b}") for b in range(bloc)]
    s_state = [nc.alloc_semaphore(f"s_state{b}") for b in range(bloc)]
    s_ocopy = [nc.alloc_semaphore(f"s_ocopy{b}") for b in range(bloc)]

    sp, pe, dve, act, pool = nc.sync, nc.tensor, nc.vector, nc.scalar, nc.gpsimd

    all_sems = (
        [s_w]
        + s_in0
        + [s for pair in s_ing for s in pair]
        + s_out0
        + [s for pair in s_og for s in pair]
        + s_mm
        + s_state
        + s_ocopy
    )
    sem_nums = sorted(s.num for s in all_sems)
    lo, hi = sem_nums[0], sem_nums[-1] + 1
    assert sem_nums == list(range(lo, hi))

    # Semaphores are per-core hardware state and are not cleared by
    # allocation: reset ours before any use, and again on exit so repeated
    # executions of this NEFF (and later kernels) see clean state.
    pool.dma_reset(range(lo, hi))
    pool.sem_clear(range(lo, hi))
    nc.all_engine_barrier()

    ing_val = [[0, 0] for _ in range(bloc)]   # slot sem value after last issued DMA
    in_need = {}                              # (cc, b) -> required slot sem value
    og_val = [[0, 0] for _ in range(bloc)]
    og_after_group = [[0] * bloc for _ in range(ngrp)]

    # ---- weights + chunk-0 inputs (SP ring) ----
    sp.dma_start(wt[:, 0, :], gw[0, :, :]).then_inc(s_w, 16)
    sp.dma_start(wt[:, 1, :], gw[1, :, :]).then_inc(s_w, 16)
    for b in range(bloc):
        n0 = min(128, t)
        sp.dma_start(rhs0[b][0:n0, :], x[b, 0:n0, :]).then_inc(s_in0[b], 16)

    # ---- chunk 0 compute ----
    pe.wait_ge(s_w, 32)
    for b in range(bloc):
        bank = 2 * b
        pe.wait_ge(s_in0[b], 16)
        pe.matmul(ps[bank][:, :], wt[:, 0, :], rhs0[b][:, :], start=True, stop=True).then_inc(
            s_mm[b], 1
        )
    for b in range(bloc):
        bank = 2 * b
        if nch > 1:
            # NOTE: state copies must run on ACT, not DVE — DVE PSUM reads
            # in this kernel shape hit a hardware error on trn2 (empirically:
            # any config with >=4 chunks and the full out path died with an
            # opaque runtime error; moving these to ACT fixed it).
            act.wait_ge(s_mm[b], 1)
            act.copy(grhs[b][0][0:2, 0, :], ps[bank][0:2, :]).then_inc(
                s_state[b], 1
            )
    for b in range(bloc):
        bank = 2 * b
        act.wait_ge(s_mm[b], 1)
        act.copy(ot0[b][:, :], ps[bank][:, :]).then_inc(s_ocopy[b], 1)
    for b in range(bloc):
        act.wait_ge(s_in0[b], 16)
        act.dma_start(out[b, 0:1, :], rhs0[b][0:1, :]).then_inc(s_out0[b], 16)
        r0 = rows_of(0)
        act.wait_ge(s_ocopy[b], 1)
        act.dma_start(out[b, 1 : 1 + r0, :], ot0[b][2 : 2 + r0, :]).then_inc(s_out0[b], 16)

    # ---- steady-state groups ----
    for g, chunks in enumerate(groups):
        slot = g % 2
        full = [cc for cc in chunks if rows_of(cc) == K]
        partial = [cc for cc in chunks if rows_of(cc) < K]
        assert full + partial == chunks
        nf = len(full)

        # input DMAs (SP ring); at most one group in flight per slot sem
        for b in range(bloc):
            if g >= 2:
                sp.wait_ge(s_mm[b], groups[g - 2][-1] + 1)
            if nf and "flat_in_dma" in debug_skip:
                for cc in full:
                    t0 = K * cc
                    j = cc - chunks[0]
                    sp.dma_start(
                        grhs[b][slot][2 : 2 + K, j, :], x[b, t0 + 1 : t0 + 1 + K, :]
                    ).then_inc(s_ing[b][slot], 16)
                    ing_val[b][slot] += 16
            elif nf:
                t0 = K * full[0]
                src = x[b, t0 + 1 : t0 + 1 + nf * K, :]
                if nf > 1:
                    src = src.rearrange("(g p) f -> p g f", g=nf)
                    dst = grhs[b][slot][2 : 2 + K, 0:nf, :]
                else:
                    dst = grhs[b][slot][2 : 2 + K, 0, :]
                sp.dma_start(dst, src).then_inc(s_ing[b][slot], 16)
                ing_val[b][slot] += 16
            for cc in partial:
                if "partial_in" in debug_skip:
                    continue
                r = rows_of(cc)
                j = cc - chunks[0]
                sp.dma_start(
                    grhs[b][slot][2 : 2 + r, j, :], x[b, K * cc + 1 : K * cc + 1 + r, :]
                ).then_inc(s_ing[b][slot], 16)
                ing_val[b][slot] += 16
            # all chunks of the group wait for the whole group's DMAs
            for cc in chunks:
                in_need[(cc, b)] = ing_val[b][slot]
                if "partial_need16" in debug_skip and partial:
                    in_need[(cc, b)] = ing_val[b][slot] - 16 * len(partial)

        # compute
        for cc in chunks:
            if cc in [p for p in partial] and "partial_mm" in debug_skip:
                continue
            j = cc - chunks[0]
            # partial (last) chunk: contract only over the rows actually
            # loaded (state + r x-rows); the dropped rows have zero weight
            # for the outputs we keep, and this avoids reading stale rows.
            kk = 2 + rows_of(cc)
            for b in range(bloc):
                bank = 2 * b + (cc & 1)
                pe.wait_ge(s_ing[b][slot], in_need[(cc, b)])
                if "no_state" not in debug_skip:
                    pe.wait_ge(s_state[b], cc)
                if "serialize" in debug_skip:
                    pe.wait_ge(s_ocopy[b], cc)
                elif cc >= 2 and "no_out" not in debug_skip:
                    pe.wait_ge(s_ocopy[b], cc - 1)
                pe.matmul(
                    ps[bank][:, :],
                    wt[0:kk, 1, :],
                    grhs[b][slot][0:kk, j, :],
                    start=True,
                    stop=True,
                ).then_inc(s_mm[b], 1)
            for b in range(bloc):
                bank = 2 * b + (cc & 1)
                if cc + 1 < nch:
                    g2 = cc // grp
                    slot2 = g2 % 2
                    j2 = cc % grp
                    act.wait_ge(s_mm[b], cc + 1)
                    act.copy(grhs[b][slot2][0:2, j2, :], ps[bank][0:2, :]).then_inc(
                        s_state[b], 1
                    )
            for b in range(bloc):
                if "no_out" in debug_skip:
                    continue
                bank = 2 * b + (cc & 1)
                if cc == chunks[0] and g >= 2:
                    act.wait_ge(s_og[b][slot], og_after_group[g - 2][b])
                act.wait_ge(s_mm[b], cc + 1)
                act.copy(gout[b][slot][:, j, :], ps[bank][:, :]).then_inc(s_ocopy[b], 1)

        # output DMAs (ACT ring)
        for b in range(bloc):
            if "no_out" in debug_skip:
                break
            last_c = full[-1] if (partial and "partial_mm" in debug_skip) else chunks[-1]
            act.wait_ge(s_ocopy[b], last_c + 1)
            if nf and "flat_out_dma" in debug_skip:
                for cc in full:
                    t0 = K * cc
                    j = cc - chunks[0]
                    act.dma_start(
                        out[b, t0 + 1 : t0 + 1 + K, :], gout[b][slot][2 : 2 + K, j, :]
                    ).then_inc(s_og[b][slot], 16)
                    og_val[b][slot] += 16
            elif nf:
                t0 = K * full[0]
                dst = out[b, t0 + 1 : t0 + 1 + nf * K, :]
                if nf > 1:
                    dst = dst.rearrange("(g p) f -> p g f", g=nf)
                    src = gout[b][slot][2 : 2 + K, 0:nf, :]
                else:
                    src = gout[b][slot][2 : 2 + K, 0, :]
                act.dma_start(dst, src).then_inc(s_og[b][slot], 16)
                og_val[b][slot] += 16
            for cc in partial:
                if "partial_out" in debug_skip or "partial_mm" in debug_skip:
                    continue
                r = rows_of(cc)
                j = cc - chunks[0]
                act.dma_start(
                    out[b, K * cc + 1 : K * cc + 1 + r, :], gout[b][slot][2 : 2 + r, j, :]
                ).then_inc(s_og[b][slot], 16)
                og_val[b][slot] += 16
            og_after_group[g][b] = og_val[b][slot]

    # ---- final: ensure all output DMAs land before program end ----
    for b in range(bloc):
        pool.wait_ge(s_out0[b], 32)
        for slot in range(2):
            if og_val[b][slot]:
                pool.wait_ge(s_og[b][slot], og_val[b][slot])
        if "final_mm_wait" in debug_skip:
            pool.wait_ge(s_mm[b], nch)

    # leave semaphores clean for the next load/execution
    pool.dma_reset(range(lo, hi))
    pool.sem_clear(range(lo, hi))

    return nc


_CACHE = {}


def _get_nc():
    if "nc" not in _CACHE:
        _CACHE["nc"] = build_nc()
    return _CACHE["nc"]


def _run(x, **kwargs):
    x = np.ascontiguousarray(np.asarray(x), dtype=np.float32)
    assert x.shape == (B, T, F), x.shape
    nc = _get_nc()
    g0t, gt = _build_gmats()
    gwv = np.ascontiguousarray(np.stack([g0t, gt]))
    in_maps = [
        {"x": np.ascontiguousarray(x[c * BLOC : (c + 1) * BLOC]), "gw": gwv}
        for c in range(NCORES)
    ]
    res = run_bass_kernel_spmd(nc, in_maps, core_ids=list(range(NCORES)), **kwargs)
    out = np.concatenate([res.results[c]["out"] for c in range(NCORES)], axis=0)
    return out, res


def kernel(x):
    return _run(x)[0]



# revision 6
# speedup vs baseline: 688.2854x; 688.2854x over previous
"""DEMA (double exponential moving average) Trainium2 kernel — blocked FIR.

Math: the per-lane recurrence
    s_t = a*x_t + (1-a)*(s_{t-1} + b_{t-1})
    b_t = B*(s_t - s_{t-1}) + (1-B)*b_{t-1}
is linear time-invariant with spectral radius sqrt(1-a) ~ 0.837, so the
impulse response decays below 1e-9 within 128 steps.  The scan is therefore
computed as a TRUNCATED FIR over two 128-step blocks: for output block c,
    out_c = H0 @ X_c + H1 @ X_{c-1}           (c >= 2)
    out_1 = H0 @ X_1 + G1 @ X_0               (G1 carries the s0/b0 init)
    out_0 = G0 @ X_0                          (row 0 of G0 is identity: s_0=x_0)
There is NO cross-block state and NO serial chain: every block is two
accumulating 128x128 @ 128x512 matmuls into one PSUM bank.

Numerics: input, weights and output are bf16 (fp32 PSUM accumulation),
halving HBM traffic.  Simulated end-to-end rel err vs the fp32 reference is
~3e-3 (tolerance 2e-2); fp32 truncation error of the 256-tap FIR is ~1e-7.

Sharding: pure data parallel over batch, 4 batches per core x 8 cores.
The host casts x to bf16 per core and casts the bf16 result back to fp32.

Engines: SP ring = input DMAs (1 MiB, 8 chunks per transfer), PE = matmuls,
DVE = PSUM->SBUF output copies for batches 0-1, ACT = copies for batches 2-3
plus all output DMAs.  Synchronization is explicit single-wait semaphores
(at most one sem wait per instruction on this toolchain).  DMA completion
sems are per buffer-slot so at most one group's DMAs are in flight per sem.
"""

import math
from contextlib import ExitStack

import numpy as np
import ml_dtypes

import concourse.bass as bass
from concourse import mybir
from concourse.bass_utils import run_bass_kernel_spmd

ALPHA = 0.3
BETA = 0.1

B, T, F = 32, 4096, 512
NCORES = 8
BLOC = B // NCORES
K = 128           # timesteps per block = matmul contraction/output size
NCH = T // K      # 32 blocks
GRP = 8           # blocks per grouped DMA (1 MiB bf16)
NGRP = NCH // GRP  # 4 groups, ping-pong over 2 slots

BF16 = mybir.dt.bfloat16
F32 = mybir.dt.float32
NP_BF16 = ml_dtypes.bfloat16

# which batches copy PSUM->SBUF on the vector engine (rest use scalar/ACT)
DVE_COPY_BATCHES = (0, 1)


def _build_mats():
    """Return (G0, G1, H1, H0) float64 [128,128]: out_c = H0@X_c + H1@X_{c-1},
    with G0/G1 handling block 0/1 init (s_0 = x_0, b_0 = x_1 - x_0)."""

    def scan(x):
        # exact reference recurrence, float64, x: [T, n] -> s_t rows incl s_0
        s = x[0].copy()
        b = x[1] - x[0]
        out = [s.copy()]
        for t in range(1, x.shape[0]):
            s_new = ALPHA * x[t] + (1 - ALPHA) * (s + b)
            b = BETA * (s_new - s) + (1 - BETA) * b
            s = s_new
            out.append(s.copy())
        return np.array(out)

    # init-aware columns: impulse at j for j in [0,128)
    imp = np.zeros((2 * K, K))
    imp[:K, :K] = np.eye(K)
    cols = scan(imp)  # [2K, K]
    G0, G1 = cols[:K], cols[K:]

    # steady impulse response (no init effects): h[k] = response at lag k
    x = np.zeros((4 * K, 1))
    J = 2 * K
    x[J, 0] = 1.0
    col = scan(x)[:, 0]
    h = col[J : J + 2 * K]
    idx_i = np.arange(K)[:, None]
    idx_j = np.arange(K)[None, :]
    lag = idx_i - idx_j
    H0 = np.where(lag >= 0, h[np.clip(lag, 0, 2 * K - 1)], 0.0)
    H1 = h[K + lag]
    return G0, G1, H1, H0


def build_nc(bloc=BLOC, t=T, f=F):
    nc = bass.Bass(enable_partition_id=False)
    st = ExitStack()
    nc._dema_exitstack = st  # keep sbuf/psum allocations alive

    nch = t // K
    ngrp = nch // GRP
    rows = GRP * K  # timesteps per group

    x = nc.dram_tensor("x", [bloc, t, f], BF16, kind="ExternalInput")
    # weights stored partition-major ([p, k, f]) so the load is one
    # contiguous-1KiB-per-partition DMA
    gw = nc.dram_tensor("gw", [128, 4, 128], BF16, kind="ExternalInput")
    out = nc.dram_tensor("out", [bloc, t, f], BF16, kind="ExternalOutput")

    ent = st.enter_context
    wt = ent(nc.sbuf_tensor("wt", [128, 4, 128], BF16))
    grhs = [
        [ent(nc.sbuf_tensor(f"grhs_{b}_{s}", [128, GRP, f], BF16)) for s in range(2)]
        for b in range(bloc)
    ]
    gout = [
        [ent(nc.sbuf_tensor(f"gout_{b}_{s}", [128, GRP, f], BF16)) for s in range(2)]
        for b in range(bloc)
    ]
    ps = [
        [ent(nc.psum_tensor(f"ps{b}_{p}", [128, f], F32)) for p in range(2)]
        for b in range(bloc)
    ]

    s_w = nc.alloc_semaphore("s_w")
    s_in = [[nc.alloc_semaphore(f"s_in{b}_{s}") for s in range(2)] for b in range(bloc)]
    s_mm = [nc.alloc_semaphore(f"s_mm{b}") for b in range(bloc)]
    s_cp = [nc.alloc_semaphore(f"s_cp{b}") for b in range(bloc)]
    s_out = [
        [nc.alloc_semaphore(f"s_out{b}_{s}") for s in range(2)] for b in range(bloc)
    ]

    sp, pe, dve, act, pool = nc.sync, nc.tensor, nc.vector, nc.scalar, nc.gpsimd

    all_sems = (
        [s_w]
        + [s for pair in s_in for s in pair]
        + s_mm
        + s_cp
        + [s for pair in s_out for s in pair]
    )
    sem_nums = sorted(s.num for s in all_sems)
    lo, hi = sem_nums[0], sem_nums[-1] + 1
    assert sem_nums == list(range(lo, hi))

    # Semaphores are per-core hardware state and are not cleared by
    # allocation: reset ours before any use, and again on exit so repeated
    # executions of this NEFF (and later kernels) see clean state.
    pool.dma_reset(range(lo, hi))
    pool.sem_clear(range(lo, hi))
    nc.all_engine_barrier()

    in_val = [[0, 0] for _ in range(bloc)]  # slot sem value after issued DMAs
    in_need = {}  # (b, g) -> required slot sem value
    out_val = [[0, 0] for _ in range(bloc)]

    # ---- weights (SP ring) ----
    sp.dma_start(wt[:, :, :], gw[:, :, :]).then_inc(s_w, 16)

    # ---- input DMAs (SP ring), gated two groups ahead ----
    def issue_in_group(g):
        slot = g % 2
        for b in range(bloc):
            if g >= 2:
                # slot tiles fully consumed once block (g-1)*GRP is done
                sp.wait_ge(s_mm[b], (g - 1) * GRP + 1)
            src = x[b, g * rows : (g + 1) * rows, :].rearrange(
                "(g p) f -> p g f", g=GRP
            )
            sp.dma_start(grhs[b][slot][:, 0:GRP, :], src).then_inc(s_in[b][slot], 16)
            in_val[b][slot] += 16
            in_need[(b, g)] = in_val[b][slot]

    issue_in_group(0)
    issue_in_group(1)

    # ---- main loop over blocks ----
    pe.wait_ge(s_w, 16)
    for cc in range(nch):
        g, j = cc // GRP, cc % GRP
        slot = g % 2

        # start-of-group bookkeeping: prefetch the group after next
        if j == 0 and g + 2 < ngrp:
            issue_in_group(g + 2)

        # matmuls: start pass (H1/G0/G1 weights), then stop pass (H0)
        for b in range(bloc):
            bank = ps[b][cc % 2]
            if cc >= 2:
                pe.wait_ge(s_cp[b], cc - 1)  # bank free (copy of cc-2 done)
            if j == 0:
                pe.wait_ge(s_in[b][slot], in_need[(b, g)])
                if cc > 0:
                    pe.wait_ge(s_in[b][1 - slot], in_need[(b, g - 1)])
            if cc == 0:
                pe.matmul(
                    bank[:, :], wt[:, 0, :], grhs[b][0][:, 0, :], start=True, stop=True
                ).then_inc(s_mm[b], 1)
            else:
                wk = 1 if cc == 1 else 2  # G1 for block 1, else H1
                pj = (cc - 1) % GRP
                pslot = ((cc - 1) // GRP) % 2
                pe.matmul(
                    bank[:, :],
                    wt[:, wk, :],
                    grhs[b][pslot][:, pj, :],
                    start=True,
                    stop=False,
                )
        if cc > 0:
            for b in range(bloc):
                bank = ps[b][cc % 2]
                pe.matmul(
                    bank[:, :], wt[:, 3, :], grhs[b][slot][:, j, :], start=False,
                    stop=True,
                ).then_inc(s_mm[b], 1)

        # PSUM -> SBUF copies (cast fp32 -> bf16)
        for b in range(bloc):
            ce = dve if b in DVE_COPY_BATCHES else act
            ce.wait_ge(s_mm[b], cc + 1)
            if j == 0 and g >= 2:
                ce.wait_ge(s_out[b][slot], 16 * (g // 2))
            if ce is act:
                ce.copy(gout[b][slot][:, j, :], ps[b][cc % 2][:, :]).then_inc(
                    s_cp[b], 1
                )
            else:
                ce.tensor_copy(gout[b][slot][:, j, :], ps[b][cc % 2][:, :]).then_inc(
                    s_cp[b], 1
                )

        # end of group: drain to HBM (ACT ring)
        if j == GRP - 1:
            for b in range(bloc):
                act.wait_ge(s_cp[b], GRP * (g + 1))
                dst = out[b, g * rows : (g + 1) * rows, :].rearrange(
                    "(g p) f -> p g f", g=GRP
                )
                act.dma_start(dst, gout[b][slot][:, 0:GRP, :]).then_inc(
                    s_out[b][slot], 16
                )
                out_val[b][slot] += 16

    # ---- final: ensure all output DMAs land before program end ----
    for b in range(bloc):
        for slot in range(2):
            if out_val[b][slot]:
                pool.wait_ge(s_out[b][slot], out_val[b][slot])

    # leave semaphores clean for the next load/execution
    pool.dma_reset(range(lo, hi))
    pool.sem_clear(range(lo, hi))

    return nc


_CACHE = {}


def _get_nc():
    if "nc" not in _CACHE:
        _CACHE["nc"] = build_nc()
    return _CACHE["nc"]


def _get_gw():
    if "gw" not in _CACHE:
        g0, g1, h1, h0 = _build_mats()
        # matmul computes lhsT.T @ rhs, so store transposed matrices; laid
        # out [p, k, f] to match the partition-major dram tensor
        _CACHE["gw"] = np.ascontiguousarray(
            np.stack([g0.T, g1.T, h1.T, h0.T]).transpose(1, 0, 2).astype(NP_BF16)
        )
    return _CACHE["gw"]


def _run(x, **kwargs):
    x = np.asarray(x)
    assert x.shape == (B, T, F), x.shape
    nc = _get_nc()
    gwv = _get_gw()
    xb = x.astype(NP_BF16)
    in_maps = [
        {"x": np.ascontiguousarray(xb[c * BLOC : (c + 1) * BLOC]), "gw": gwv}
        for c in range(NCORES)
    ]
    res = run_bass_kernel_spmd(nc, in_maps, core_ids=list(range(NCORES)), **kwargs)
    out = np.concatenate(
        [np.asarray(res.results[c]["out"]).astype(np.float32) for c in range(NCORES)],
        axis=0,
    )
    return out, res


def kernel(x):
    return _run(x)[0]


# revision 11
# speedup vs baseline: 712.9926x; 1.0359x over previous
"""DEMA (double exponential moving average) Trainium2 kernel — blocked FIR.

Math: the per-lane recurrence
    s_t = a*x_t + (1-a)*(s_{t-1} + b_{t-1})
    b_t = B*(s_t - s_{t-1}) + (1-B)*b_{t-1}
is linear time-invariant with spectral radius sqrt(1-a) ~ 0.837, so the
impulse response decays below 1e-9 within 128 steps.  The scan is therefore
computed as a TRUNCATED FIR over two 128-step blocks: for output block c,
    out_c = H0 @ X_c + H1 @ X_{c-1}           (c >= 2)
    out_1 = H0 @ X_1 + G1 @ X_0               (G1 carries the s0/b0 init)
    out_0 = G0 @ X_0                          (row 0 of G0 is identity: s_0=x_0)
There is NO cross-block state and NO serial chain: every block is two
accumulating 128x128 @ 128x512 matmuls into one PSUM bank.

Numerics: input, weights and output are bf16 (fp32 PSUM accumulation),
halving HBM traffic.  Simulated end-to-end rel err vs the fp32 reference is
~3e-3 (tolerance 2e-2); fp32 truncation error of the 256-tap FIR is ~1e-7.

Sharding: pure data parallel over batch, 4 batches per core x 8 cores.
The host casts x to bf16 per core and casts the bf16 result back to fp32.

Engines: SP ring = input DMAs (1 MiB, 8 chunks per transfer), PE = matmuls,
DVE = PSUM->SBUF output copies for batches 0-1, ACT = copies for batches 2-3
plus all output DMAs.  Synchronization is explicit single-wait semaphores
(at most one sem wait per instruction on this toolchain).  DMA completion
sems are per buffer-slot so at most one group's DMAs are in flight per sem.
"""

import math
from contextlib import ExitStack

import numpy as np
import ml_dtypes

import concourse.bass as bass
from concourse import mybir
from concourse.bass_utils import run_bass_kernel_spmd

ALPHA = 0.3
BETA = 0.1

B, T, F = 32, 4096, 512
NCORES = 8
BLOC = B // NCORES
K = 128           # timesteps per block = matmul contraction/output size
NCH = T // K      # 32 blocks
GRP = 8           # blocks per grouped DMA (1 MiB bf16)
NGRP = NCH // GRP  # 4 groups
NSLOT = 3         # input buffer slots (only the last group is gated)

BF16 = mybir.dt.bfloat16
F32 = mybir.dt.float32
NP_BF16 = ml_dtypes.bfloat16

# which batches copy PSUM->SBUF on the vector engine (rest use scalar/ACT,
# which also issues the output DMAs)
DVE_COPY_BATCHES = (0, 1, 2)


def _build_mats():
    """Return (G0, G1, H1, H0) float64 [128,128]: out_c = H0@X_c + H1@X_{c-1},
    with G0/G1 handling block 0/1 init (s_0 = x_0, b_0 = x_1 - x_0)."""

    def scan(x):
        # exact reference recurrence, float64, x: [T, n] -> s_t rows incl s_0
        s = x[0].copy()
        b = x[1] - x[0]
        out = [s.copy()]
        for t in range(1, x.shape[0]):
            s_new = ALPHA * x[t] + (1 - ALPHA) * (s + b)
            b = BETA * (s_new - s) + (1 - BETA) * b
            s = s_new
            out.append(s.copy())
        return np.array(out)

    # init-aware columns: impulse at j for j in [0,128)
    imp = np.zeros((2 * K, K))
    imp[:K, :K] = np.eye(K)
    cols = scan(imp)  # [2K, K]
    G0, G1 = cols[:K], cols[K:]

    # steady impulse response (no init effects): h[k] = response at lag k
    x = np.zeros((4 * K, 1))
    J = 2 * K
    x[J, 0] = 1.0
    col = scan(x)[:, 0]
    h = col[J : J + 2 * K]
    idx_i = np.arange(K)[:, None]
    idx_j = np.arange(K)[None, :]
    lag = idx_i - idx_j
    H0 = np.where(lag >= 0, h[np.clip(lag, 0, 2 * K - 1)], 0.0)
    H1 = h[K + lag]
    return G0, G1, H1, H0


def build_nc(bloc=BLOC, t=T, f=F):
    nc = bass.Bass(enable_partition_id=False)
    st = ExitStack()
    nc._dema_exitstack = st  # keep sbuf/psum allocations alive

    nch = t // K
    ngrp = nch // GRP
    rows = GRP * K  # timesteps per group

    x = nc.dram_tensor("x", [bloc, t, f], BF16, kind="ExternalInput")
    # weights stored partition-major and FLAT ([p, 4*128]) so the load is one
    # contiguous-1KiB-per-partition DMA (128 descriptors, fast HWDGE issue)
    gw = nc.dram_tensor("gw", [128, 4 * 128], BF16, kind="ExternalInput")
    out = nc.dram_tensor("out", [bloc, t, f], BF16, kind="ExternalOutput")

    ent = st.enter_context
    wt = ent(nc.sbuf_tensor("wt", [128, 4 * 128], BF16))
    grhs = [
        [
            ent(nc.sbuf_tensor(f"grhs_{b}_{s}", [128, GRP, f], BF16))
            for s in range(NSLOT)
        ]
        for b in range(bloc)
    ]
    gout = [
        [ent(nc.sbuf_tensor(f"gout_{b}_{s}", [128, GRP, f], BF16)) for s in range(2)]
        for b in range(bloc)
    ]
    ps = [
        [ent(nc.psum_tensor(f"ps{b}_{p}", [128, f], F32)) for p in range(2)]
        for b in range(bloc)
    ]

    s_w = nc.alloc_semaphore("s_w")
    s_in = [
        [nc.alloc_semaphore(f"s_in{b}_{s}") for s in range(NSLOT)] for b in range(bloc)
    ]
    s_mm = [nc.alloc_semaphore(f"s_mm{b}") for b in range(bloc)]
    s_cp = [nc.alloc_semaphore(f"s_cp{b}") for b in range(bloc)]
    s_out = [
        [nc.alloc_semaphore(f"s_out{b}_{s}") for s in range(2)] for b in range(bloc)
    ]

    sp, pe, dve, act, pool = nc.sync, nc.tensor, nc.vector, nc.scalar, nc.gpsimd

    all_sems = (
        [s_w]
        + [s for pair in s_in for s in pair]
        + s_mm
        + s_cp
        + [s for pair in s_out for s in pair]
    )
    sem_nums = sorted(s.num for s in all_sems)
    lo, hi = sem_nums[0], sem_nums[-1] + 1
    assert sem_nums == list(range(lo, hi))

    # Semaphores are per-core hardware state and are not cleared by
    # allocation: reset ours before any use, and again on exit so repeated
    # executions of this NEFF (and later kernels) see clean state.
    pool.dma_reset(range(lo, hi))
    pool.sem_clear(range(lo, hi))
    nc.all_engine_barrier()

    in_val = [[0] * NSLOT for _ in range(bloc)]  # slot sem value after DMAs
    in_need = {}  # (b, g) -> required slot sem value
    out_val = [[0, 0] for _ in range(bloc)]

    # ---- input + weight DMAs (SP ring) ----
    def issue_in_group(g):
        slot = g % NSLOT
        for b in range(bloc):
            if g >= NSLOT:
                # slot tiles fully consumed once block (g-2)*GRP is done
                sp.wait_ge(s_mm[b], (g - 2) * GRP + 1)
            src = x[b, g * rows : (g + 1) * rows, :].rearrange(
                "(g p) f -> p g f", g=GRP
            )
            sp.dma_start(grhs[b][slot][:, 0:GRP, :], src).then_inc(s_in[b][slot], 16)
            in_val[b][slot] += 16
            in_need[(b, g)] = in_val[b][slot]

    issue_in_group(0)
    sp.dma_start(wt[:, :], gw[:, :]).then_inc(s_w, 16)
    for g in range(1, NSLOT):
        issue_in_group(g)

    # ---- main loop over blocks ----
    pe.wait_ge(s_w, 16)
    for cc in range(nch):
        g, j = cc // GRP, cc % GRP
        slot = g % NSLOT

        # start-of-group bookkeeping: prefetch NSLOT groups ahead
        if j == 0 and g + NSLOT < ngrp:
            issue_in_group(g + NSLOT)

        # matmuls: start pass (H1/G0/G1 weights), then stop pass (H0)
        for b in range(bloc):
            bank = ps[b][cc % 2]
            if cc >= 2:
                pe.wait_ge(s_cp[b], cc - 1)  # bank free (copy of cc-2 done)
            if j == 0:
                pe.wait_ge(s_in[b][slot], in_need[(b, g)])
                if cc > 0:
                    pslot = (g - 1) % NSLOT
                    pe.wait_ge(s_in[b][pslot], in_need[(b, g - 1)])
            if cc == 0:
                pe.matmul(
                    bank[:, :],
                    wt[:, 0:128],
                    grhs[b][0][:, 0, :],
                    start=True,
                    stop=True,
                ).then_inc(s_mm[b], 1)
            else:
                wk = 1 if cc == 1 else 2  # G1 for block 1, else H1
                pj = (cc - 1) % GRP
                pslot = ((cc - 1) // GRP) % NSLOT
                pe.matmul(
                    bank[:, :],
                    wt[:, 128 * wk : 128 * (wk + 1)],
                    grhs[b][pslot][:, pj, :],
                    start=True,
                    stop=False,
                )
        if cc > 0:
            for b in range(bloc):
                bank = ps[b][cc % 2]
                pe.matmul(
                    bank[:, :],
                    wt[:, 384:512],
                    grhs[b][slot][:, j, :],
                    start=False,
                    stop=True,
                ).then_inc(s_mm[b], 1)

        # PSUM -> SBUF copies (cast fp32 -> bf16)
        oslot = g % 2
        for b in range(bloc):
            ce = dve if b in DVE_COPY_BATCHES else act
            ce.wait_ge(s_mm[b], cc + 1)
            if j == 0 and g >= 2:
                ce.wait_ge(s_out[b][oslot], 16 * (g // 2))
            if ce is act:
                ce.copy(gout[b][oslot][:, j, :], ps[b][cc % 2][:, :]).then_inc(
                    s_cp[b], 1
                )
            else:
                ce.tensor_copy(gout[b][oslot][:, j, :], ps[b][cc % 2][:, :]).then_inc(
                    s_cp[b], 1
                )

        # output drain to HBM (ACT ring): full group at j==7, except the last
        # group which drains in halves (j==3 and j==7) to shorten the tail
        half = GRP // 2
        if g == ngrp - 1 and j == half - 1:
            for b in range(bloc):
                act.wait_ge(s_cp[b], GRP * g + half)
                dst = out[b, g * rows : g * rows + half * K, :].rearrange(
                    "(g p) f -> p g f", g=half
                )
                act.dma_start(dst, gout[b][oslot][:, 0:half, :]).then_inc(
                    s_out[b][oslot], 16
                )
                out_val[b][oslot] += 16
        if j == GRP - 1:
            for b in range(bloc):
                act.wait_ge(s_cp[b], GRP * (g + 1))
                if g == ngrp - 1:
                    dst = out[
                        b, g * rows + half * K : (g + 1) * rows, :
                    ].rearrange("(g p) f -> p g f", g=half)
                    act.dma_start(dst, gout[b][oslot][:, half:GRP, :]).then_inc(
                        s_out[b][oslot], 16
                    )
                else:
                    dst = out[b, g * rows : (g + 1) * rows, :].rearrange(
                        "(g p) f -> p g f", g=GRP
                    )
                    act.dma_start(dst, gout[b][oslot][:, 0:GRP, :]).then_inc(
                        s_out[b][oslot], 16
                    )
                out_val[b][oslot] += 16

    # ---- final: ensure all output DMAs land before program end ----
    for b in range(bloc):
        for slot in range(2):
            if out_val[b][slot]:
                pool.wait_ge(s_out[b][slot], out_val[b][slot])

    # leave semaphores clean for the next load/execution
    pool.dma_reset(range(lo, hi))
    pool.sem_clear(range(lo, hi))

    return nc


_CACHE = {}


def _get_nc():
    if "nc" not in _CACHE:
        _CACHE["nc"] = build_nc()
    return _CACHE["nc"]


def _get_gw():
    if "gw" not in _CACHE:
        g0, g1, h1, h0 = _build_mats()
        # matmul computes lhsT.T @ rhs, so store transposed matrices; laid
        # out partition-major and flattened to [p, 4*128]
        _CACHE["gw"] = np.ascontiguousarray(
            np.stack([g0.T, g1.T, h1.T, h0.T])
            .transpose(1, 0, 2)
            .reshape(128, 4 * 128)
            .astype(NP_BF16)
        )
    return _CACHE["gw"]


def _run(x, **kwargs):
    x = np.asarray(x)
    assert x.shape == (B, T, F), x.shape
    nc = _get_nc()
    gwv = _get_gw()
    xb = x.astype(NP_BF16)
    in_maps = [
        {"x": np.ascontiguousarray(xb[c * BLOC : (c + 1) * BLOC]), "gw": gwv}
        for c in range(NCORES)
    ]
    res = run_bass_kernel_spmd(nc, in_maps, core_ids=list(range(NCORES)), **kwargs)
    out = np.concatenate(
        [np.asarray(res.results[c]["out"]).astype(np.float32) for c in range(NCORES)],
        axis=0,
    )
    return out, res


def kernel(x):
    return _run(x)[0]
